# revision 4
# baseline (speedup 1.0000x reference)
"""EpipolarSampler TRN2 kernel.

Strategy
--------
The heavy output is `feat` (b,v,1,r,s,c) = bilinear grid-samples of the
pair-view image at 2M points x 128 channels (256 MB fp32).  All per-ray
projection math (tiny) runs on host exactly as the reference does; the
bilinear sampling runs on 8 NeuronCores as one SPMD Bass kernel.

Device formulation: for a tile of 128 points (64 rays x 2 samples) all
sample taps fall in a 4-image-row window (empirically; the builder widens
uniformly if a dataset needs it).  The bilinear sample then becomes
PSUM-accumulated one-hot matmuls:

    psum[pt, ch] = sum_j W_j[k, pt]^T @ img_rows_j[k, ch],  k = 128 pixels

with host-built sparse-in-dense fp16 weight matrices W (4 taps/column,
overlap and bounds masks folded in) and host-sliced image-row windows.
Both stream from HBM; addresses are fully static so one program serves
all 8 cores (shard = (b, v, ray-half); per-core data differs only).
"""
import sys
sys.path.insert(0, '/opt/trn_rl_repo')

import numpy as np

NUM_SAMPLES = 32
EPS = 1e-6

# problem geometry (from the task spec)
B, V, C, H, W = 2, 2, 128, 64, 64
R = H * W                      # rays per view
S = NUM_SAMPLES
NCORES = 8
RAYS_PER_CORE = R // 2         # 2048: shard (b, v, ray-half)
TILE_RAYS = 64                 # rays per tile
TILE_S = 2                     # samples per tile
NRB = RAYS_PER_CORE // TILE_RAYS    # 32 ray blocks
NSP = S // TILE_S                   # 16 sample pairs
NTILES = NSP * NRB                  # 512 tiles/core, t = sp*NRB + rb


# --------------------------------------------------------------------------
# host-side reference math (verbatim port of the reference module, minus the
# grid sample) — run with jax on CPU for bit-compatible small outputs.
# --------------------------------------------------------------------------

def _small_outputs(images, extrinsics, intrinsics, near, far):
    import jax
    import jax.numpy as jnp

    cpu = jax.local_devices(backend="cpu")[0]

    def _het_indices(n):
        v = np.arange(n)[:, None]
        ov = np.arange(n - 1)[None, :]
        index_v = ov + (ov >= v)
        t_v = index_v.copy()
        t_ov = v - (v > ov)
        return jnp.asarray(index_v), jnp.asarray(t_v), jnp.asarray(t_ov)

    def _hom_pt(p):
        return jnp.concatenate([p, jnp.ones_like(p[..., :1])], axis=-1)

    def _hom_vec(v):
        return jnp.concatenate([v, jnp.zeros_like(v[..., :1])], axis=-1)

    def _intersect_image_coordinate(K, o, d, dim, cv):
        od = 1 - dim
        g = lambda i, j: K[..., i, j][..., None]
        fs, fo, cs, co = g(dim, dim), g(od, od), g(dim, 2), g(od, 2)
        os_, oo, oz = o[..., dim], o[..., od], o[..., 2]
        ds, do, dz = d[..., dim], d[..., od], d[..., 2]
        c = (cv - cs) / fs
        t = (c * oz - os_) / (ds - c * dz)
        den = ds * oz - dz * os_
        den = jnp.where(den == 0.0, EPS, den)
        other = co + fo * (oo * (ds - c * dz) + do * (c * oz - os_)) / den
        same = jnp.full_like(other, cv)
        xy = jnp.stack([same, other] if dim == 0 else [other, same], axis=-1)
        valid = ((oz + t * dz) > EPS) & (t > EPS)
        return t, xy, valid

    def _reduce(projs, is_min):
        t = jnp.stack([p[0] for p in projs], axis=-1)
        xy = jnp.stack([p[1] for p in projs], axis=-2)
        valid = jnp.stack([p[2] for p in projs], axis=-1)
        worst = jnp.inf if is_min else -jnp.inf
        tm = jnp.where(valid, t, worst)
        sel = (jnp.argmin if is_min else jnp.argmax)(tm, axis=-1)
        xy_sel = jnp.take_along_axis(xy, sel[..., None, None], axis=-2)[..., 0, :]
        v_sel = jnp.take_along_axis(valid, sel[..., None], axis=-1)[..., 0]
        return xy_sel, v_sel

    def _point_projection(o, d, tval, K):
        pt = o + tval[..., None] * d
        p = pt / (pt[..., 2:3] + 1e-8)
        p = jnp.nan_to_num(p, nan=0.0, posinf=1e8, neginf=-1e8)
        xy = jnp.einsum('bnoij,bnorj->bnori', K, p)[..., :2]
        tb = jnp.broadcast_to(tval, pt.shape[:-1])
        valid = (pt[..., 2] > EPS) & (tb > EPS)
        return xy, valid

    def _project_rays(origins, directions, ext_o, K_o, near, far):
        w2c = jnp.linalg.inv(ext_o)
        o = jnp.einsum('bnoij,bnrj->bnori', w2c, _hom_pt(origins))[..., :3]
        d = jnp.einsum('bnoij,bnrj->bnori', w2c, _hom_vec(directions))[..., :3]
        fi = [_intersect_image_coordinate(K_o, o, d, 0, 0.0),
              _intersect_image_coordinate(K_o, o, d, 0, 1.0),
              _intersect_image_coordinate(K_o, o, d, 1, 0.0),
              _intersect_image_coordinate(K_o, o, d, 1, 1.0)]
        fmin_xy, fmin_ok = _reduce(fi, True)
        fmax_xy, fmax_ok = _reduce(fi, False)
        near_xy, near_ok = _point_projection(o, d, near[:, :, None, None], K_o)
        far_xy, far_ok = _point_projection(o, d, far[:, :, None, None], K_o)
        xy_min = jnp.where(near_ok[..., None], near_xy, fmin_xy)
        xy_max = jnp.where(far_ok[..., None], far_xy, fmax_xy)
        overlaps = (near_ok | fmin_ok) & (far_ok | fmax_ok)
        return xy_min, xy_max, overlaps

    with jax.default_device(cpu):
        images = jnp.asarray(images)
        extrinsics = jnp.asarray(extrinsics)
        intrinsics = jnp.asarray(intrinsics)
        near = jnp.asarray(near)
        far = jnp.asarray(far)

        b, v, c, h, w = images.shape
        r, s = h * w, NUM_SAMPLES
        index_v, t_v, t_ov = _het_indices(v)
        xg = (jnp.arange(w, dtype=images.dtype) + 0.5) / w
        yg = (jnp.arange(h, dtype=images.dtype) + 0.5) / h
        xy = jnp.stack(jnp.meshgrid(xg, yg, indexing='xy'), axis=-1).reshape(r, 2)
        d_cam = jnp.einsum('bnij,rj->bnri', jnp.linalg.inv(intrinsics), _hom_pt(xy))
        d_cam = d_cam / jnp.linalg.norm(d_cam, axis=-1, keepdims=True)
        directions = jnp.einsum('bnij,bnrj->bnri', extrinsics[..., :3, :3], d_cam)
        origins = jnp.broadcast_to(extrinsics[:, :, None, :3, 3], directions.shape)
        ext_o = jnp.take(extrinsics, index_v, axis=1)
        K_o = jnp.take(intrinsics, index_v, axis=1)
        xy_min, xy_max, overlaps = _project_rays(origins, directions, ext_o, K_o,
                                                 near, far)
        ov_f = overlaps.astype(images.dtype)
        xy_min = jnp.nan_to_num(xy_min, nan=0.0, posinf=0.0, neginf=0.0) * ov_f[..., None]
        xy_max = jnp.nan_to_num(xy_max, nan=0.0, posinf=0.0, neginf=0.0) * ov_f[..., None]
        depth = ((jnp.arange(s, dtype=images.dtype) + 0.5) / s)[:, None]
        lo = xy_min[..., None, :]
        delta = xy_max[..., None, :] - lo
        xy_sample = lo + depth * delta
        half = 0.5 / s
        xy_sample_near = lo + (depth - half) * delta
        xy_sample_far = lo + (depth + half) * delta
        xy_ray = jnp.broadcast_to(xy[None, None], (b, v, r, 2))

    return (np.asarray(overlaps), np.asarray(xy_ray), np.asarray(xy_sample),
            np.asarray(xy_sample_near), np.asarray(xy_sample_far),
            np.asarray(origins), np.asarray(directions))


# --------------------------------------------------------------------------
# per-core stream builders
# --------------------------------------------------------------------------

def _build_streams(imgT16, g, ov, nchunks):
    """imgT16: [4096, 128] fp16 sampled image (pixel-major);
    g: [2048, 32, 2] fp32 sample coords in [0,1] for this core's rays;
    ov: [2048] bool overlap for this core's rays.
    Returns (w_stream [128, NTILES*nchunks*128] f16,
             r_stream [128, NTILES*nchunks*128] f16)."""
    win = 2 * nchunks                     # window height in image rows
    px = g[..., 0] * W - 0.5              # [2048, 32]
    py = g[..., 1] * H - 0.5
    x0 = np.floor(px); y0 = np.floor(py)
    wx1 = (px - x0).astype(np.float32); wy1 = (py - y0).astype(np.float32)
    wx0 = 1.0 - wx1; wy0 = 1.0 - wy1
    x0 = x0.astype(np.int64); y0 = y0.astype(np.int64)

    ovm = ov[:, None].astype(np.float32)          # [2048, 1]

    # per-point tile/partition indices
    ray = np.arange(RAYS_PER_CORE)
    rb, ri = ray // TILE_RAYS, ray % TILE_RAYS
    samp = np.arange(S)
    sp, so = samp // TILE_S, samp % TILE_S
    tile = sp[None, :] * NRB + rb[:, None]         # [2048, 32]
    part = so[None, :] * TILE_RAYS + ri[:, None]   # [2048, 32] point-in-tile

    # contributing rows per tap (for anchors): a tap (ty, tx) contributes iff
    # 0<=ty<H, 0<=tx<W and its weight is nonzero-able; anchor from y-extent.
    taps = []
    for dy, wy in ((0, wy0), (1, wy1)):
        for dx, wx in ((0, wx0), (1, wx1)):
            ty = y0 + dy; tx = x0 + dx
            wgt = wy * wx * ovm
            valid = (ty >= 0) & (ty < H) & (tx >= 0) & (tx < W) & (wgt != 0.0)
            taps.append((ty, tx, wgt, valid))

    # anchors per tile from contributing-row extent
    INF = np.int64(1 << 30)
    ymin = np.full((NTILES,), INF); ymax = np.full((NTILES,), np.int64(-1))
    tflat = tile.ravel()
    for ty, tx, wgt, valid in taps:
        tyf = ty.ravel(); vf = valid.ravel()
        tv = tflat[vf]
        np.minimum.at(ymin, tv, tyf[vf])
        np.maximum.at(ymax, tv, tyf[vf])
    ymin = np.where(ymin == INF, 0, ymin)
    ymax = np.maximum(ymax, ymin)
    span = int((ymax - ymin + 1).max()) if NTILES else 1
    assert span <= win, f"tile row-span {span} exceeds window {win}"
    anchors = np.clip(ymin, 0, H - win)            # [NTILES]

    # W stream scatter
    ncols = NTILES * nchunks * 128
    w_stream = np.zeros((128, ncols), np.float16)
    wf = w_stream.ravel()
    a_of_pt = anchors[tflat]                       # [pts]
    pf = part.ravel()
    for ty, tx, wgt, valid in taps:
        vf = valid.ravel()
        if not vf.any():
            continue
        tyv = ty.ravel()[vf]; txv = tx.ravel()[vf]
        av = a_of_pt[vf]; tv = tflat[vf]; pv = pf[vf]
        k = (tyv - av) * W + txv                   # [0, win*64)
        assert (k >= 0).all() and (k < win * 64).all()
        chunk = k >> 7
        kin = k & 127
        col = tv * (nchunks * 128) + chunk * 128 + pv
        flat = kin * ncols + col
        wf[flat] = wf[flat] + wgt.ravel()[vf].astype(np.float16)

    # rhs stream gather: tile t chunk j = image rows [a+2j, a+2j+2) = 128 pix
    bases = (anchors[:, None] + 2 * np.arange(nchunks)[None, :]) * W   # [NT, nc]
    idx = bases.reshape(-1, 1) + np.arange(128)[None, :]               # [NT*nc, 128]
    blocks = imgT16[idx, :]                        # [NT*nc, 128 pix, 128 ch]
    r_stream = np.ascontiguousarray(
        blocks.transpose(1, 0, 2).reshape(128, ncols))
    return w_stream, r_stream


# --------------------------------------------------------------------------
# bass program (built once per nchunks value)
# --------------------------------------------------------------------------

def _split_multi_waits(nc, mybir):
    # this container's walrus rejects >1 sync-wait per instruction; spread
    # extra waits across single-wait nops on the same engine.
    for f in nc.m.functions:
        for bb in f.blocks:
            new = []
            for inst in bb.instructions:
                si = inst.sync_info
                waits = list(si.on_wait) if si is not None and si.on_wait else []
                if len(waits) > 1:
                    for w in waits[:-1]:
                        nop = mybir.InstNoOp(
                            name=f"I-waitsplit-{nc.next_id()}", ins=[], outs=[])
                        nop.engine = inst.engine
                        nop.sync_info = mybir.SyncInfo(on_wait=[w], on_update=[])
                        new.append(nop)
                    si.on_wait = waits[-1:]
                new.append(inst)
            bb.instructions = new


def _build_program(nchunks):
    import concourse.bass as bass
    import concourse.mybir as mybir
    from concourse.tile import TileContext

    FP16, FP32 = mybir.dt.float16, mybir.dt.float32
    CW = nchunks * 128                     # stream columns per tile
    nc = bass.Bass(trn_type="TRN2")
    w = nc.dram_tensor("w", [128, NTILES * CW], FP16, kind="ExternalInput")
    r = nc.dram_tensor("r", [128, NTILES * CW], FP16, kind="ExternalInput")
    out = nc.dram_tensor("out", [RAYS_PER_CORE, S, C], FP32, kind="ExternalOutput")

    with TileContext(nc) as tc:
        with (
            tc.tile_pool(name="wseg", bufs=3) as wpool,
            tc.tile_pool(name="rseg", bufs=3) as rpool,
            tc.tile_pool(name="res", bufs=2) as respool,
            tc.tile_pool(name="ps", bufs=8, space="PSUM") as ppool,
        ):
            for sp in range(NSP):
                seg0 = sp * NRB * CW
                wt = wpool.tile([128, NRB * CW], FP16, tag="w")
                rt = rpool.tile([128, NRB * CW], FP16, tag="r")
                nc.sync.dma_start(wt[:], w[:, seg0:seg0 + NRB * CW])
                nc.sync.dma_start(rt[:], r[:, seg0:seg0 + NRB * CW])
                res = respool.tile([128, NRB * C], FP32, tag="res")
                for rb in range(NRB):
                    ps = ppool.tile([128, C], FP32)
                    for j in range(nchunks):
                        o = rb * CW + j * 128
                        nc.tensor.matmul(ps[:], lhsT=wt[:, o:o + 128],
                                         rhs=rt[:, o:o + 128],
                                         start=(j == 0), stop=(j == nchunks - 1))
                    dst = res[:, rb * C:(rb + 1) * C]
                    if rb % 5 < 3:
                        nc.vector.tensor_copy(dst, ps[:])
                    else:
                        nc.scalar.copy(dst, ps[:])
                # out[ray, sp*2+so, :]; src partitions p = so*64 + ri
                for so in range(TILE_S):
                    dst = out[:, sp * TILE_S + so, :].rearrange(
                        "(rb ri) ch -> ri rb ch", rb=NRB)
                    src = res[so * TILE_RAYS:(so + 1) * TILE_RAYS, :].rearrange(
                        "p (rb ch) -> p rb ch", ch=C)
                    nc.sync.dma_start(dst, src)
    _split_multi_waits(nc, mybir)
    return nc


# --------------------------------------------------------------------------
# entry point
# --------------------------------------------------------------------------

def kernel(images, extrinsics, intrinsics, near, far):
    from concourse import bass_utils

    images = np.asarray(images)
    extrinsics = np.asarray(extrinsics, np.float32)
    intrinsics = np.asarray(intrinsics, np.float32)
    near = np.asarray(near, np.float32)
    far = np.asarray(far, np.float32)
    b, v, c, h, w_ = images.shape
    assert (b, v, c, h, w_) == (B, V, C, H, W), "kernel specialized to spec shape"

    (overlaps, xy_ray, xy_sample, xy_sample_near, xy_sample_far,
     origins, directions) = _small_outputs(images, extrinsics, intrinsics,
                                           near, far)

    # sampling coords for image (b,i) come from the pair view (v=2: 1-i)
    # final feat[b,i,0] = bilinear(images[b,1-i], xy_sample[b,i,0]) * ov[b,i]
    imgT16 = {}
    for bi in range(B):
        for vi in range(V):
            imgT16[(bi, vi)] = np.ascontiguousarray(
                images[bi, vi].reshape(C, H * W).T).astype(np.float16)

    # per-core assignments: core = (bi*V + vi)*2 + half
    core_meta = []
    for bi in range(B):
        for vi in range(V):
            for half in range(2):
                r0 = half * RAYS_PER_CORE
                g = xy_sample[bi, vi, 0, r0:r0 + RAYS_PER_CORE]      # [2048,32,2]
                ov = overlaps[bi, vi, 0, r0:r0 + RAYS_PER_CORE]      # [2048]
                core_meta.append((bi, vi, r0, g, ov))

    # find uniform chunk count (max row-span over all cores' tiles)
    nchunks = 2
    built = []
    while True:
        try:
            built = [
                _build_streams(imgT16[(bi, 1 - vi)], g, ov, nchunks)
                for (bi, vi, r0, g, ov) in core_meta
            ]
            break
        except AssertionError:
            nchunks += 1
            if nchunks > H // 2:
                raise

    nc = _build_program(nchunks)
    in_maps = [{"w": ws, "r": rs} for (ws, rs) in built]
    res = bass_utils.run_bass_kernel_spmd(
        nc, in_maps, core_ids=list(range(NCORES)), trace=False)

    feat = np.empty((B, V, 1, R, S, C), np.float32)
    for ci, (bi, vi, r0, g, ov) in enumerate(core_meta):
        feat[bi, vi, 0, r0:r0 + RAYS_PER_CORE] = res.results[ci]["out"]

    return (feat, overlaps, xy_ray, xy_sample, xy_sample_near, xy_sample_far,
            origins, directions)


# revision 5
# speedup vs baseline: 1.0141x; 1.0141x over previous
"""EpipolarSampler TRN2 kernel.

Strategy
--------
The heavy output is `feat` (b,v,1,r,s,c) = bilinear grid-samples of the
pair-view image at 2M points x 128 channels (256 MB fp32).  All per-ray
projection math (tiny) runs on host exactly as the reference does; the
bilinear sampling runs on 8 NeuronCores as one SPMD Bass kernel.

Device formulation: for a tile of 128 points (64 rays x 2 samples) all
sample taps fall in a 4-image-row window (empirically; the builder widens
uniformly if a dataset needs it).  The bilinear sample then becomes
PSUM-accumulated one-hot matmuls:

    psum[pt, ch] = sum_j W_j[k, pt]^T @ img_rows_j[k, ch],  k = 128 pixels

with host-built sparse-in-dense fp16 weight matrices W (4 taps/column,
overlap and bounds masks folded in) and host-sliced image-row windows.
Both stream from HBM; addresses are fully static so one program serves
all 8 cores (shard = (b, v, ray-half); per-core data differs only).
"""
import sys
sys.path.insert(0, '/opt/trn_rl_repo')

import numpy as np

NUM_SAMPLES = 32
EPS = 1e-6

# problem geometry (from the task spec)
B, V, C, H, W = 2, 2, 128, 64, 64
R = H * W                      # rays per view
S = NUM_SAMPLES
NCORES = 8
RAYS_PER_CORE = R // 2         # 2048: shard (b, v, ray-half)
TILE_RAYS = 64                 # rays per tile
TILE_S = 2                     # samples per tile
NRB = RAYS_PER_CORE // TILE_RAYS    # 32 ray blocks
NSP = S // TILE_S                   # 16 sample pairs
NTILES = NSP * NRB                  # 512 tiles/core, t = sp*NRB + rb


# --------------------------------------------------------------------------
# host-side reference math (verbatim port of the reference module, minus the
# grid sample) — run with jax on CPU for bit-compatible small outputs.
# --------------------------------------------------------------------------

def _small_outputs(images, extrinsics, intrinsics, near, far):
    import jax
    import jax.numpy as jnp

    cpu = jax.local_devices(backend="cpu")[0]

    def _het_indices(n):
        v = np.arange(n)[:, None]
        ov = np.arange(n - 1)[None, :]
        index_v = ov + (ov >= v)
        t_v = index_v.copy()
        t_ov = v - (v > ov)
        return jnp.asarray(index_v), jnp.asarray(t_v), jnp.asarray(t_ov)

    def _hom_pt(p):
        return jnp.concatenate([p, jnp.ones_like(p[..., :1])], axis=-1)

    def _hom_vec(v):
        return jnp.concatenate([v, jnp.zeros_like(v[..., :1])], axis=-1)

    def _intersect_image_coordinate(K, o, d, dim, cv):
        od = 1 - dim
        g = lambda i, j: K[..., i, j][..., None]
        fs, fo, cs, co = g(dim, dim), g(od, od), g(dim, 2), g(od, 2)
        os_, oo, oz = o[..., dim], o[..., od], o[..., 2]
        ds, do, dz = d[..., dim], d[..., od], d[..., 2]
        c = (cv - cs) / fs
        t = (c * oz - os_) / (ds - c * dz)
        den = ds * oz - dz * os_
        den = jnp.where(den == 0.0, EPS, den)
        other = co + fo * (oo * (ds - c * dz) + do * (c * oz - os_)) / den
        same = jnp.full_like(other, cv)
        xy = jnp.stack([same, other] if dim == 0 else [other, same], axis=-1)
        valid = ((oz + t * dz) > EPS) & (t > EPS)
        return t, xy, valid

    def _reduce(projs, is_min):
        t = jnp.stack([p[0] for p in projs], axis=-1)
        xy = jnp.stack([p[1] for p in projs], axis=-2)
        valid = jnp.stack([p[2] for p in projs], axis=-1)
        worst = jnp.inf if is_min else -jnp.inf
        tm = jnp.where(valid, t, worst)
        sel = (jnp.argmin if is_min else jnp.argmax)(tm, axis=-1)
        xy_sel = jnp.take_along_axis(xy, sel[..., None, None], axis=-2)[..., 0, :]
        v_sel = jnp.take_along_axis(valid, sel[..., None], axis=-1)[..., 0]
        return xy_sel, v_sel

    def _point_projection(o, d, tval, K):
        pt = o + tval[..., None] * d
        p = pt / (pt[..., 2:3] + 1e-8)
        p = jnp.nan_to_num(p, nan=0.0, posinf=1e8, neginf=-1e8)
        xy = jnp.einsum('bnoij,bnorj->bnori', K, p)[..., :2]
        tb = jnp.broadcast_to(tval, pt.shape[:-1])
        valid = (pt[..., 2] > EPS) & (tb > EPS)
        return xy, valid

    def _project_rays(origins, directions, ext_o, K_o, near, far):
        w2c = jnp.linalg.inv(ext_o)
        o = jnp.einsum('bnoij,bnrj->bnori', w2c, _hom_pt(origins))[..., :3]
        d = jnp.einsum('bnoij,bnrj->bnori', w2c, _hom_vec(directions))[..., :3]
        fi = [_intersect_image_coordinate(K_o, o, d, 0, 0.0),
              _intersect_image_coordinate(K_o, o, d, 0, 1.0),
              _intersect_image_coordinate(K_o, o, d, 1, 0.0),
              _intersect_image_coordinate(K_o, o, d, 1, 1.0)]
        fmin_xy, fmin_ok = _reduce(fi, True)
        fmax_xy, fmax_ok = _reduce(fi, False)
        near_xy, near_ok = _point_projection(o, d, near[:, :, None, None], K_o)
        far_xy, far_ok = _point_projection(o, d, far[:, :, None, None], K_o)
        xy_min = jnp.where(near_ok[..., None], near_xy, fmin_xy)
        xy_max = jnp.where(far_ok[..., None], far_xy, fmax_xy)
        overlaps = (near_ok | fmin_ok) & (far_ok | fmax_ok)
        return xy_min, xy_max, overlaps

    with jax.default_device(cpu):
        images = jnp.asarray(images)
        extrinsics = jnp.asarray(extrinsics)
        intrinsics = jnp.asarray(intrinsics)
        near = jnp.asarray(near)
        far = jnp.asarray(far)

        b, v, c, h, w = images.shape
        r, s = h * w, NUM_SAMPLES
        index_v, t_v, t_ov = _het_indices(v)
        xg = (jnp.arange(w, dtype=images.dtype) + 0.5) / w
        yg = (jnp.arange(h, dtype=images.dtype) + 0.5) / h
        xy = jnp.stack(jnp.meshgrid(xg, yg, indexing='xy'), axis=-1).reshape(r, 2)
        d_cam = jnp.einsum('bnij,rj->bnri', jnp.linalg.inv(intrinsics), _hom_pt(xy))
        d_cam = d_cam / jnp.linalg.norm(d_cam, axis=-1, keepdims=True)
        directions = jnp.einsum('bnij,bnrj->bnri', extrinsics[..., :3, :3], d_cam)
        origins = jnp.broadcast_to(extrinsics[:, :, None, :3, 3], directions.shape)
        ext_o = jnp.take(extrinsics, index_v, axis=1)
        K_o = jnp.take(intrinsics, index_v, axis=1)
        xy_min, xy_max, overlaps = _project_rays(origins, directions, ext_o, K_o,
                                                 near, far)
        ov_f = overlaps.astype(images.dtype)
        xy_min = jnp.nan_to_num(xy_min, nan=0.0, posinf=0.0, neginf=0.0) * ov_f[..., None]
        xy_max = jnp.nan_to_num(xy_max, nan=0.0, posinf=0.0, neginf=0.0) * ov_f[..., None]
        depth = ((jnp.arange(s, dtype=images.dtype) + 0.5) / s)[:, None]
        lo = xy_min[..., None, :]
        delta = xy_max[..., None, :] - lo
        xy_sample = lo + depth * delta
        half = 0.5 / s
        xy_sample_near = lo + (depth - half) * delta
        xy_sample_far = lo + (depth + half) * delta
        xy_ray = jnp.broadcast_to(xy[None, None], (b, v, r, 2))

    return (np.asarray(overlaps), np.asarray(xy_ray), np.asarray(xy_sample),
            np.asarray(xy_sample_near), np.asarray(xy_sample_far),
            np.asarray(origins), np.asarray(directions))


# --------------------------------------------------------------------------
# per-core stream builders
# --------------------------------------------------------------------------

def _build_streams(imgT16, g, ov, nchunks):
    """imgT16: [4096, 128] fp16 sampled image (pixel-major);
    g: [2048, 32, 2] fp32 sample coords in [0,1] for this core's rays;
    ov: [2048] bool overlap for this core's rays.
    Returns (w_stream [128, NTILES*nchunks*128] f16,
             r_stream [128, NTILES*nchunks*128] f16)."""
    win = 2 * nchunks                     # window height in image rows
    px = g[..., 0] * W - 0.5              # [2048, 32]
    py = g[..., 1] * H - 0.5
    x0 = np.floor(px); y0 = np.floor(py)
    wx1 = (px - x0).astype(np.float32); wy1 = (py - y0).astype(np.float32)
    wx0 = 1.0 - wx1; wy0 = 1.0 - wy1
    x0 = x0.astype(np.int64); y0 = y0.astype(np.int64)

    ovm = ov[:, None].astype(np.float32)          # [2048, 1]

    # per-point tile/partition indices
    ray = np.arange(RAYS_PER_CORE)
    rb, ri = ray // TILE_RAYS, ray % TILE_RAYS
    samp = np.arange(S)
    sp, so = samp // TILE_S, samp % TILE_S
    tile = sp[None, :] * NRB + rb[:, None]         # [2048, 32]
    part = so[None, :] * TILE_RAYS + ri[:, None]   # [2048, 32] point-in-tile

    # contributing rows per tap (for anchors): a tap (ty, tx) contributes iff
    # 0<=ty<H, 0<=tx<W and its weight is nonzero-able; anchor from y-extent.
    taps = []
    for dy, wy in ((0, wy0), (1, wy1)):
        for dx, wx in ((0, wx0), (1, wx1)):
            ty = y0 + dy; tx = x0 + dx
            wgt = wy * wx * ovm
            valid = (ty >= 0) & (ty < H) & (tx >= 0) & (tx < W) & (wgt != 0.0)
            taps.append((ty, tx, wgt, valid))

    # anchors per tile from contributing-row extent
    INF = np.int64(1 << 30)
    ymin = np.full((NTILES,), INF); ymax = np.full((NTILES,), np.int64(-1))
    tflat = tile.ravel()
    for ty, tx, wgt, valid in taps:
        tyf = ty.ravel(); vf = valid.ravel()
        tv = tflat[vf]
        np.minimum.at(ymin, tv, tyf[vf])
        np.maximum.at(ymax, tv, tyf[vf])
    ymin = np.where(ymin == INF, 0, ymin)
    ymax = np.maximum(ymax, ymin)
    span = int((ymax - ymin + 1).max()) if NTILES else 1
    assert span <= win, f"tile row-span {span} exceeds window {win}"
    anchors = np.clip(ymin, 0, H - win)            # [NTILES]

    # W stream scatter
    ncols = NTILES * nchunks * 128
    w_stream = np.zeros((128, ncols), np.float16)
    wf = w_stream.ravel()
    a_of_pt = anchors[tflat]                       # [pts]
    pf = part.ravel()
    for ty, tx, wgt, valid in taps:
        vf = valid.ravel()
        if not vf.any():
            continue
        tyv = ty.ravel()[vf]; txv = tx.ravel()[vf]
        av = a_of_pt[vf]; tv = tflat[vf]; pv = pf[vf]
        k = (tyv - av) * W + txv                   # [0, win*64)
        assert (k >= 0).all() and (k < win * 64).all()
        chunk = k >> 7
        kin = k & 127
        col = tv * (nchunks * 128) + chunk * 128 + pv
        flat = kin * ncols + col
        wf[flat] = wf[flat] + wgt.ravel()[vf].astype(np.float16)

    # rhs stream gather: tile t chunk j = image rows [a+2j, a+2j+2) = 128 pix
    bases = (anchors[:, None] + 2 * np.arange(nchunks)[None, :]) * W   # [NT, nc]
    idx = bases.reshape(-1, 1) + np.arange(128)[None, :]               # [NT*nc, 128]
    blocks = imgT16[idx, :]                        # [NT*nc, 128 pix, 128 ch]
    r_stream = np.ascontiguousarray(
        blocks.transpose(1, 0, 2).reshape(128, ncols))
    return w_stream, r_stream


# --------------------------------------------------------------------------
# bass program (built once per nchunks value)
# --------------------------------------------------------------------------

def _split_multi_waits(nc, mybir):
    # this container's walrus rejects >1 sync-wait per instruction; spread
    # extra waits across single-wait nops on the same engine.
    for f in nc.m.functions:
        for bb in f.blocks:
            new = []
            for inst in bb.instructions:
                si = inst.sync_info
                waits = list(si.on_wait) if si is not None and si.on_wait else []
                if len(waits) > 1:
                    for w in waits[:-1]:
                        nop = mybir.InstNoOp(
                            name=f"I-waitsplit-{nc.next_id()}", ins=[], outs=[])
                        nop.engine = inst.engine
                        nop.sync_info = mybir.SyncInfo(on_wait=[w], on_update=[])
                        new.append(nop)
                    si.on_wait = waits[-1:]
                new.append(inst)
            bb.instructions = new


def _build_program(nchunks):
    import concourse.bass as bass
    import concourse.mybir as mybir
    from concourse.tile import TileContext

    FP16, FP32 = mybir.dt.float16, mybir.dt.float32
    CW = nchunks * 128                     # stream columns per tile
    nc = bass.Bass(trn_type="TRN2")
    w = nc.dram_tensor("w", [128, NTILES * CW], FP16, kind="ExternalInput")
    r = nc.dram_tensor("r", [128, NTILES * CW], FP16, kind="ExternalInput")
    out = nc.dram_tensor("out", [RAYS_PER_CORE, S, C], FP16, kind="ExternalOutput")

    with TileContext(nc) as tc:
        with (
            tc.tile_pool(name="wseg", bufs=3) as wpool,
            tc.tile_pool(name="rseg", bufs=3) as rpool,
            tc.tile_pool(name="res", bufs=2) as respool,
            tc.tile_pool(name="ps", bufs=8, space="PSUM") as ppool,
        ):
            for sp in range(NSP):
                seg0 = sp * NRB * CW
                wt = wpool.tile([128, NRB * CW], FP16, tag="w")
                rt = rpool.tile([128, NRB * CW], FP16, tag="r")
                nc.sync.dma_start(wt[:], w[:, seg0:seg0 + NRB * CW])
                nc.sync.dma_start(rt[:], r[:, seg0:seg0 + NRB * CW])
                res = respool.tile([128, NRB * C], FP16, tag="res")
                for rb in range(NRB):
                    ps = ppool.tile([128, C], FP32)
                    for j in range(nchunks):
                        o = rb * CW + j * 128
                        nc.tensor.matmul(ps[:], lhsT=wt[:, o:o + 128],
                                         rhs=rt[:, o:o + 128],
                                         start=(j == 0), stop=(j == nchunks - 1))
                    dst = res[:, rb * C:(rb + 1) * C]
                    if rb % 5 < 3:
                        nc.vector.tensor_copy(dst, ps[:])
                    else:
                        nc.scalar.copy(dst, ps[:])
                # out[ray, sp*2+so, :]; src partitions p = so*64 + ri
                for so in range(TILE_S):
                    dst = out[:, sp * TILE_S + so, :].rearrange(
                        "(rb ri) ch -> ri rb ch", rb=NRB)
                    src = res[so * TILE_RAYS:(so + 1) * TILE_RAYS, :].rearrange(
                        "p (rb ch) -> p rb ch", ch=C)
                    nc.sync.dma_start(dst, src)
    _split_multi_waits(nc, mybir)
    return nc


# --------------------------------------------------------------------------
# entry point
# --------------------------------------------------------------------------

def kernel(images, extrinsics, intrinsics, near, far):
    from concourse import bass_utils

    images = np.asarray(images)
    extrinsics = np.asarray(extrinsics, np.float32)
    intrinsics = np.asarray(intrinsics, np.float32)
    near = np.asarray(near, np.float32)
    far = np.asarray(far, np.float32)
    b, v, c, h, w_ = images.shape
    assert (b, v, c, h, w_) == (B, V, C, H, W), "kernel specialized to spec shape"

    (overlaps, xy_ray, xy_sample, xy_sample_near, xy_sample_far,
     origins, directions) = _small_outputs(images, extrinsics, intrinsics,
                                           near, far)

    # sampling coords for image (b,i) come from the pair view (v=2: 1-i)
    # final feat[b,i,0] = bilinear(images[b,1-i], xy_sample[b,i,0]) * ov[b,i]
    imgT16 = {}
    for bi in range(B):
        for vi in range(V):
            imgT16[(bi, vi)] = np.ascontiguousarray(
                images[bi, vi].reshape(C, H * W).T).astype(np.float16)

    # per-core assignments: core = (bi*V + vi)*2 + half
    core_meta = []
    for bi in range(B):
        for vi in range(V):
            for half in range(2):
                r0 = half * RAYS_PER_CORE
                g = xy_sample[bi, vi, 0, r0:r0 + RAYS_PER_CORE]      # [2048,32,2]
                ov = overlaps[bi, vi, 0, r0:r0 + RAYS_PER_CORE]      # [2048]
                core_meta.append((bi, vi, r0, g, ov))

    # find uniform chunk count (max row-span over all cores' tiles)
    nchunks = 2
    built = []
    while True:
        try:
            built = [
                _build_streams(imgT16[(bi, 1 - vi)], g, ov, nchunks)
                for (bi, vi, r0, g, ov) in core_meta
            ]
            break
        except AssertionError:
            nchunks += 1
            if nchunks > H // 2:
                raise

    nc = _build_program(nchunks)
    in_maps = [{"w": ws, "r": rs} for (ws, rs) in built]
    res = bass_utils.run_bass_kernel_spmd(
        nc, in_maps, core_ids=list(range(NCORES)), trace=False)

    feat = np.empty((B, V, 1, R, S, C), np.float32)
    for ci, (bi, vi, r0, g, ov) in enumerate(core_meta):
        feat[bi, vi, 0, r0:r0 + RAYS_PER_CORE] = res.results[ci]["out"]

    return (feat, overlaps, xy_ray, xy_sample, xy_sample_near, xy_sample_far,
            origins, directions)


# revision 8
# speedup vs baseline: 1.3816x; 1.3624x over previous
"""EpipolarSampler TRN2 kernel.

Strategy
--------
The heavy output is `feat` (b,v,1,r,s,c) = bilinear grid-samples of the
pair-view image at 2M points x 128 channels (256 MB fp32).  All per-ray
projection math (tiny) runs on host exactly as the reference does; the
bilinear sampling runs on 8 NeuronCores as one SPMD Bass kernel.

Device formulation: for a tile of 128 points (64 rays x 2 samples) all
sample taps fall in a 4-image-row window (empirically; the builder widens
uniformly if a dataset needs it).  The bilinear sample then becomes
PSUM-accumulated one-hot matmuls:

    psum[pt, ch] = sum_j W_j[k, pt]^T @ img_rows_j[k, ch],  k = 128 pixels

with host-built sparse-in-dense fp16 weight matrices W (4 taps/column,
overlap and bounds masks folded in) and host-sliced image-row windows.
Both stream from HBM; addresses are fully static so one program serves
all 8 cores (shard = (b, v, ray-half); per-core data differs only).
"""
import sys
sys.path.insert(0, '/opt/trn_rl_repo')

import numpy as np

NUM_SAMPLES = 32
EPS = 1e-6

# problem geometry (from the task spec)
B, V, C, H, W = 2, 2, 128, 64, 64
R = H * W                      # rays per view
S = NUM_SAMPLES
NCORES = 8
RAYS_PER_CORE = R // 2         # 2048: shard (b, v, ray-half)
TILE_RAYS = 64                 # rays per tile
TILE_S = 2                     # samples per tile
NRB = RAYS_PER_CORE // TILE_RAYS    # 32 ray blocks
NSP = S // TILE_S                   # 16 sample pairs
NTILES = NSP * NRB                  # 512 tiles/core, t = sp*NRB + rb


# --------------------------------------------------------------------------
# host-side reference math (verbatim port of the reference module, minus the
# grid sample) — run with jax on CPU for bit-compatible small outputs.
# --------------------------------------------------------------------------

def _small_outputs(images, extrinsics, intrinsics, near, far):
    import jax
    import jax.numpy as jnp

    cpu = jax.local_devices(backend="cpu")[0]

    def _het_indices(n):
        v = np.arange(n)[:, None]
        ov = np.arange(n - 1)[None, :]
        index_v = ov + (ov >= v)
        t_v = index_v.copy()
        t_ov = v - (v > ov)
        return jnp.asarray(index_v), jnp.asarray(t_v), jnp.asarray(t_ov)

    def _hom_pt(p):
        return jnp.concatenate([p, jnp.ones_like(p[..., :1])], axis=-1)

    def _hom_vec(v):
        return jnp.concatenate([v, jnp.zeros_like(v[..., :1])], axis=-1)

    def _intersect_image_coordinate(K, o, d, dim, cv):
        od = 1 - dim
        g = lambda i, j: K[..., i, j][..., None]
        fs, fo, cs, co = g(dim, dim), g(od, od), g(dim, 2), g(od, 2)
        os_, oo, oz = o[..., dim], o[..., od], o[..., 2]
        ds, do, dz = d[..., dim], d[..., od], d[..., 2]
        c = (cv - cs) / fs
        t = (c * oz - os_) / (ds - c * dz)
        den = ds * oz - dz * os_
        den = jnp.where(den == 0.0, EPS, den)
        other = co + fo * (oo * (ds - c * dz) + do * (c * oz - os_)) / den
        same = jnp.full_like(other, cv)
        xy = jnp.stack([same, other] if dim == 0 else [other, same], axis=-1)
        valid = ((oz + t * dz) > EPS) & (t > EPS)
        return t, xy, valid

    def _reduce(projs, is_min):
        t = jnp.stack([p[0] for p in projs], axis=-1)
        xy = jnp.stack([p[1] for p in projs], axis=-2)
        valid = jnp.stack([p[2] for p in projs], axis=-1)
        worst = jnp.inf if is_min else -jnp.inf
        tm = jnp.where(valid, t, worst)
        sel = (jnp.argmin if is_min else jnp.argmax)(tm, axis=-1)
        xy_sel = jnp.take_along_axis(xy, sel[..., None, None], axis=-2)[..., 0, :]
        v_sel = jnp.take_along_axis(valid, sel[..., None], axis=-1)[..., 0]
        return xy_sel, v_sel

    def _point_projection(o, d, tval, K):
        pt = o + tval[..., None] * d
        p = pt / (pt[..., 2:3] + 1e-8)
        p = jnp.nan_to_num(p, nan=0.0, posinf=1e8, neginf=-1e8)
        xy = jnp.einsum('bnoij,bnorj->bnori', K, p)[..., :2]
        tb = jnp.broadcast_to(tval, pt.shape[:-1])
        valid = (pt[..., 2] > EPS) & (tb > EPS)
        return xy, valid

    def _project_rays(origins, directions, ext_o, K_o, near, far):
        w2c = jnp.linalg.inv(ext_o)
        o = jnp.einsum('bnoij,bnrj->bnori', w2c, _hom_pt(origins))[..., :3]
        d = jnp.einsum('bnoij,bnrj->bnori', w2c, _hom_vec(directions))[..., :3]
        fi = [_intersect_image_coordinate(K_o, o, d, 0, 0.0),
              _intersect_image_coordinate(K_o, o, d, 0, 1.0),
              _intersect_image_coordinate(K_o, o, d, 1, 0.0),
              _intersect_image_coordinate(K_o, o, d, 1, 1.0)]
        fmin_xy, fmin_ok = _reduce(fi, True)
        fmax_xy, fmax_ok = _reduce(fi, False)
        near_xy, near_ok = _point_projection(o, d, near[:, :, None, None], K_o)
        far_xy, far_ok = _point_projection(o, d, far[:, :, None, None], K_o)
        xy_min = jnp.where(near_ok[..., None], near_xy, fmin_xy)
        xy_max = jnp.where(far_ok[..., None], far_xy, fmax_xy)
        overlaps = (near_ok | fmin_ok) & (far_ok | fmax_ok)
        return xy_min, xy_max, overlaps

    with jax.default_device(cpu):
        images = jnp.asarray(images)
        extrinsics = jnp.asarray(extrinsics)
        intrinsics = jnp.asarray(intrinsics)
        near = jnp.asarray(near)
        far = jnp.asarray(far)

        b, v, c, h, w = images.shape
        r, s = h * w, NUM_SAMPLES
        index_v, t_v, t_ov = _het_indices(v)
        xg = (jnp.arange(w, dtype=images.dtype) + 0.5) / w
        yg = (jnp.arange(h, dtype=images.dtype) + 0.5) / h
        xy = jnp.stack(jnp.meshgrid(xg, yg, indexing='xy'), axis=-1).reshape(r, 2)
        d_cam = jnp.einsum('bnij,rj->bnri', jnp.linalg.inv(intrinsics), _hom_pt(xy))
        d_cam = d_cam / jnp.linalg.norm(d_cam, axis=-1, keepdims=True)
        directions = jnp.einsum('bnij,bnrj->bnri', extrinsics[..., :3, :3], d_cam)
        origins = jnp.broadcast_to(extrinsics[:, :, None, :3, 3], directions.shape)
        ext_o = jnp.take(extrinsics, index_v, axis=1)
        K_o = jnp.take(intrinsics, index_v, axis=1)
        xy_min, xy_max, overlaps = _project_rays(origins, directions, ext_o, K_o,
                                                 near, far)
        ov_f = overlaps.astype(images.dtype)
        xy_min = jnp.nan_to_num(xy_min, nan=0.0, posinf=0.0, neginf=0.0) * ov_f[..., None]
        xy_max = jnp.nan_to_num(xy_max, nan=0.0, posinf=0.0, neginf=0.0) * ov_f[..., None]
        depth = ((jnp.arange(s, dtype=images.dtype) + 0.5) / s)[:, None]
        lo = xy_min[..., None, :]
        delta = xy_max[..., None, :] - lo
        xy_sample = lo + depth * delta
        half = 0.5 / s
        xy_sample_near = lo + (depth - half) * delta
        xy_sample_far = lo + (depth + half) * delta
        xy_ray = jnp.broadcast_to(xy[None, None], (b, v, r, 2))

    return (np.asarray(overlaps), np.asarray(xy_ray), np.asarray(xy_sample),
            np.asarray(xy_sample_near), np.asarray(xy_sample_far),
            np.asarray(origins), np.asarray(directions))


# --------------------------------------------------------------------------
# per-core stream builders
# --------------------------------------------------------------------------

def _build_streams(imgT16, g, ov, nchunks):
    """imgT16: [4096, 128] fp16 sampled image (pixel-major);
    g: [2048, 32, 2] fp32 sample coords in [0,1] for this core's rays;
    ov: [2048] bool overlap for this core's rays.
    Returns (w_stream [128, NTILES*nchunks*128] f16,
             r_stream [128, NTILES*nchunks*128] f16)."""
    win = 2 * nchunks                     # window height in image rows
    px = g[..., 0] * W - 0.5              # [2048, 32]
    py = g[..., 1] * H - 0.5
    x0 = np.floor(px); y0 = np.floor(py)
    wx1 = (px - x0).astype(np.float32); wy1 = (py - y0).astype(np.float32)
    wx0 = 1.0 - wx1; wy0 = 1.0 - wy1
    x0 = x0.astype(np.int64); y0 = y0.astype(np.int64)

    ovm = ov[:, None].astype(np.float32)          # [2048, 1]

    # per-point tile/partition indices
    ray = np.arange(RAYS_PER_CORE)
    rb, ri = ray // TILE_RAYS, ray % TILE_RAYS
    samp = np.arange(S)
    sp, so = samp // TILE_S, samp % TILE_S
    tile = sp[None, :] * NRB + rb[:, None]         # [2048, 32]
    part = so[None, :] * TILE_RAYS + ri[:, None]   # [2048, 32] point-in-tile

    # contributing rows per tap (for anchors): a tap (ty, tx) contributes iff
    # 0<=ty<H, 0<=tx<W and its weight is nonzero-able; anchor from y-extent.
    taps = []
    for dy, wy in ((0, wy0), (1, wy1)):
        for dx, wx in ((0, wx0), (1, wx1)):
            ty = y0 + dy; tx = x0 + dx
            wgt = wy * wx * ovm
            valid = (ty >= 0) & (ty < H) & (tx >= 0) & (tx < W) & (wgt != 0.0)
            taps.append((ty, tx, wgt, valid))

    # anchors per tile from contributing-row extent
    INF = np.int64(1 << 30)
    ymin = np.full((NTILES,), INF); ymax = np.full((NTILES,), np.int64(-1))
    tflat = tile.ravel()
    for ty, tx, wgt, valid in taps:
        tyf = ty.ravel(); vf = valid.ravel()
        tv = tflat[vf]
        np.minimum.at(ymin, tv, tyf[vf])
        np.maximum.at(ymax, tv, tyf[vf])
    ymin = np.where(ymin == INF, 0, ymin)
    ymax = np.maximum(ymax, ymin)
    span = int((ymax - ymin + 1).max()) if NTILES else 1
    assert span <= win, f"tile row-span {span} exceeds window {win}"
    anchors = np.clip(ymin, 0, H - win)            # [NTILES]

    # W stream scatter
    ncols = NTILES * nchunks * 128
    w_stream = np.zeros((128, ncols), np.float16)
    wf = w_stream.ravel()
    a_of_pt = anchors[tflat]                       # [pts]
    pf = part.ravel()
    for ty, tx, wgt, valid in taps:
        vf = valid.ravel()
        if not vf.any():
            continue
        tyv = ty.ravel()[vf]; txv = tx.ravel()[vf]
        av = a_of_pt[vf]; tv = tflat[vf]; pv = pf[vf]
        k = (tyv - av) * W + txv                   # [0, win*64)
        assert (k >= 0).all() and (k < win * 64).all()
        chunk = k >> 7
        kin = k & 127
        col = tv * (nchunks * 128) + chunk * 128 + pv
        flat = kin * ncols + col
        wf[flat] = wf[flat] + wgt.ravel()[vf].astype(np.float16)

    # rhs stream gather: tile t chunk j = image rows [a+2j, a+2j+2) = 128 pix
    bases = (anchors[:, None] + 2 * np.arange(nchunks)[None, :]) * W   # [NT, nc]
    idx = bases.reshape(-1, 1) + np.arange(128)[None, :]               # [NT*nc, 128]
    blocks = imgT16[idx, :]                        # [NT*nc, 128 pix, 128 ch]
    r_stream = np.ascontiguousarray(
        blocks.transpose(1, 0, 2).reshape(128, ncols))
    return w_stream, r_stream


# --------------------------------------------------------------------------
# bass program (built once per nchunks value)
# --------------------------------------------------------------------------

def _split_multi_waits(nc, mybir):
    # this container's walrus rejects >1 sync-wait per instruction; spread
    # extra waits across single-wait nops on the same engine.
    for f in nc.m.functions:
        for bb in f.blocks:
            new = []
            for inst in bb.instructions:
                si = inst.sync_info
                waits = list(si.on_wait) if si is not None and si.on_wait else []
                if len(waits) > 1:
                    for w in waits[:-1]:
                        nop = mybir.InstNoOp(
                            name=f"I-waitsplit-{nc.next_id()}", ins=[], outs=[])
                        nop.engine = inst.engine
                        nop.sync_info = mybir.SyncInfo(on_wait=[w], on_update=[])
                        new.append(nop)
                    si.on_wait = waits[-1:]
                new.append(inst)
            bb.instructions = new


def _build_program(nchunks):
    import concourse.bass as bass
    import concourse.mybir as mybir
    from concourse.tile import TileContext

    FP16, FP32 = mybir.dt.float16, mybir.dt.float32
    CW = nchunks * 128                     # stream columns per tile
    nc = bass.Bass(trn_type="TRN2")
    w = nc.dram_tensor("w", [128, NTILES * CW], FP16, kind="ExternalInput")
    r = nc.dram_tensor("r", [128, NTILES * CW], FP16, kind="ExternalInput")
    # raw staging-buffer dump [sp][p = so*64+ri][rb][ch]; host un-permutes
    out = nc.dram_tensor("out", [NSP, 128, NRB * C], FP16, kind="ExternalOutput")

    with TileContext(nc) as tc:
        with (
            tc.tile_pool(name="wseg", bufs=3) as wpool,
            tc.tile_pool(name="rseg", bufs=3) as rpool,
            tc.tile_pool(name="res", bufs=2) as respool,
            tc.tile_pool(name="ps", bufs=8, space="PSUM") as ppool,
        ):
            for sp in range(NSP):
                seg0 = sp * NRB * CW
                wt = wpool.tile([128, NRB * CW], FP16, tag="w")
                rt = rpool.tile([128, NRB * CW], FP16, tag="r")
                nc.sync.dma_start(wt[:], w[:, seg0:seg0 + NRB * CW])
                nc.sync.dma_start(rt[:], r[:, seg0:seg0 + NRB * CW])
                res = respool.tile([128, NRB * C], FP16, tag="res")
                for rb in range(NRB):
                    ps = ppool.tile([128, C], FP32)
                    for j in range(nchunks):
                        o = rb * CW + j * 128
                        nc.tensor.matmul(ps[:], lhsT=wt[:, o:o + 128],
                                         rhs=rt[:, o:o + 128],
                                         start=(j == 0), stop=(j == nchunks - 1))
                    dst = res[:, rb * C:(rb + 1) * C]
                    if rb % 5 < 3:
                        nc.vector.tensor_copy(dst, ps[:])
                    else:
                        nc.scalar.copy(dst, ps[:])
                nc.sync.dma_start(out[sp], res[:])
    _split_multi_waits(nc, mybir)
    return nc


# --------------------------------------------------------------------------
# entry point
# --------------------------------------------------------------------------

def kernel(images, extrinsics, intrinsics, near, far):
    from concourse import bass_utils

    images = np.asarray(images)
    extrinsics = np.asarray(extrinsics, np.float32)
    intrinsics = np.asarray(intrinsics, np.float32)
    near = np.asarray(near, np.float32)
    far = np.asarray(far, np.float32)
    b, v, c, h, w_ = images.shape
    assert (b, v, c, h, w_) == (B, V, C, H, W), "kernel specialized to spec shape"

    (overlaps, xy_ray, xy_sample, xy_sample_near, xy_sample_far,
     origins, directions) = _small_outputs(images, extrinsics, intrinsics,
                                           near, far)

    # sampling coords for image (b,i) come from the pair view (v=2: 1-i)
    # final feat[b,i,0] = bilinear(images[b,1-i], xy_sample[b,i,0]) * ov[b,i]
    imgT16 = {}
    for bi in range(B):
        for vi in range(V):
            imgT16[(bi, vi)] = np.ascontiguousarray(
                images[bi, vi].reshape(C, H * W).T).astype(np.float16)

    # per-core assignments: core = (bi*V + vi)*2 + half
    core_meta = []
    for bi in range(B):
        for vi in range(V):
            for half in range(2):
                r0 = half * RAYS_PER_CORE
                g = xy_sample[bi, vi, 0, r0:r0 + RAYS_PER_CORE]      # [2048,32,2]
                ov = overlaps[bi, vi, 0, r0:r0 + RAYS_PER_CORE]      # [2048]
                core_meta.append((bi, vi, r0, g, ov))

    # find uniform chunk count (max row-span over all cores' tiles)
    nchunks = 2
    built = []
    while True:
        try:
            built = [
                _build_streams(imgT16[(bi, 1 - vi)], g, ov, nchunks)
                for (bi, vi, r0, g, ov) in core_meta
            ]
            break
        except AssertionError:
            nchunks += 1
            if nchunks > H // 2:
                raise

    nc = _build_program(nchunks)
    in_maps = [{"w": ws, "r": rs} for (ws, rs) in built]
    res = bass_utils.run_bass_kernel_spmd(
        nc, in_maps, core_ids=list(range(NCORES)), trace=False)

    feat = np.empty((B, V, 1, R, S, C), np.float32)
    for ci, (bi, vi, r0, g, ov) in enumerate(core_meta):
        raw = res.results[ci]["out"]                    # [NSP, 128, NRB*C] f16
        raw = raw.reshape(NSP, TILE_S, TILE_RAYS, NRB, C)   # sp, so, ri, rb, ch
        feat[bi, vi, 0, r0:r0 + RAYS_PER_CORE] = (
            raw.transpose(3, 2, 0, 1, 4).reshape(RAYS_PER_CORE, S, C))

    return (feat, overlaps, xy_ray, xy_sample, xy_sample_near, xy_sample_far,
            origins, directions)
